# revision 54
# baseline (speedup 1.0000x reference)
"""Trainium2 Bass kernel for nn_LyotFilter: out = x @ w_norm(weight_).

Strategy (data-parallel over 8 NeuronCores, memory-bound):
  - Host: compute the tiny [200, 64] normalized filter matrix in float32
    (mimicking the reference's f32 arithmetic), cast to bf16, and reshape
    each core's row-shard of x into a transposed, contiguous bf16
    [200, 62500] layout so the contraction dim lands on SBUF partitions
    with fully contiguous per-partition DMA.  bf16 wire format halves HBM
    traffic vs f32 (33 MB/core vs 66 MB); measured end-to-end absmax-rel
    error ~3.9e-3 (gate 2e-2).
  - Device (per core): stream xT chunks HBM->SBUF on the two HWDGE rings
    (t1 on sync, t2 on scalar), TensorE matmul (K split 128+72, PSUM fp32
    accumulate, 512-col blocks, 4 banks per PSUM tile), drain PSUM->SBUF
    with bf16 downcast alternating Vector/Scalar, and stream out.T back
    on the two rings, alternating, so ring-serial transfer time stays
    balanced.  Output placement alternates per chunk between partition
    halves 64:128 (odd SDMA ports) and a split pair layout on 0:64 +
    64:128 (both parities) -- the 50/50 mix minimizes the max per-port
    DMA byte load given that t1 covers all 16 ports while t2 (72
    partitions) is even-port-heavy.  Input DMAs are emitted two chunks
    ahead of compute so ring issues are never queued behind copy waits
    in an engine's FIFO.
  - Host: concatenate the 8 [64, 62500] results, transpose, cast f32.
"""

import functools

import numpy as np

N_CORES = 8
ROWS = 500000
RPC = ROWS // N_CORES  # 62500 rows per core
IN_DIM = 200
OUT_DIM = 64
K1 = 128               # first contraction chunk (partition limit)
K2 = IN_DIM - K1       # 72
PGROUP = 2048          # columns per PSUM tile (4 banks)
INNER = 512            # matmul moving free dim (one fp32 PSUM bank)
# chunk schedule: small leading chunks cut the pipeline-fill latency;
# the natural remainder tail shortens the drain (sums to RPC)
SCHED = [6144, 6144] + [8192] * 6 + [1060]
F_MAX = max(SCHED)
assert sum(SCHED) == RPC


def _w_norm(weight_: np.ndarray) -> np.ndarray:
    """[200, 64] filter matrix, float32 arithmetic mimicking the reference."""
    n = np.arange(220)
    skip = ((n >= 103) & (n <= 107)) | ((n >= 149) & (n <= 162)) | (n == 219)
    kept = n[~skip]
    bands = (400.0 + (2500.0 - 400.0) * kept / 220.0).astype(np.float32)
    num = np.float32(2.0 * np.pi * (-0.01))
    denom = weight_.astype(np.float32)[:, None] * (bands[None, :] * np.float32(1e-6))
    phase = (num / denom).astype(np.float32)
    w = np.float32(0.5) - np.float32(0.5) * np.cos(phase)
    wt = w.T  # [200, 64]
    wn = np.maximum(wt, np.float32(0.0)) / wt.sum(axis=0, dtype=np.float32)
    return np.ascontiguousarray(wn.astype(np.float32))


@functools.cache
def _build():
    import concourse.bass as bass
    import concourse.tile as tile
    from concourse import bacc, mybir

    f32 = mybir.dt.float32
    bf16 = mybir.dt.bfloat16
    nc = bacc.Bacc(
        "TRN2", target_bir_lowering=False, debug=False, num_devices=N_CORES
    )
    xt = nc.declare_dram_parameter("xt", [IN_DIM, RPC], bf16, isOutput=False)
    wn = nc.declare_dram_parameter("wn", [IN_DIM, OUT_DIM], bf16, isOutput=False)
    out = nc.declare_dram_parameter("out_t", [OUT_DIM, RPC], bf16, isOutput=True)

    starts = np.cumsum([0] + SCHED[:-1]).tolist()
    ncopy = 0

    with tile.TileContext(nc) as tc:
        with (
            tc.tile_pool(name="w", bufs=1) as wp,
            tc.tile_pool(name="xt1", bufs=4) as p1,
            tc.tile_pool(name="xt2", bufs=4) as p2,
            tc.tile_pool(name="outp", bufs=4) as po,
            tc.tile_pool(name="ps", bufs=2, space=bass.MemorySpace.PSUM) as pp,
        ):
            # weights ride SWDGE so the HWDGE rings start on x immediately
            w1 = wp.tile([K1, OUT_DIM], bf16, tag="w1")
            w2 = wp.tile([K2, OUT_DIM], bf16, tag="w2")
            nc.gpsimd.dma_start(w1[:], wn[0:K1, :])
            nc.gpsimd.dma_start(w2[:], wn[K1:IN_DIM, :])

            def issue_loads(ci):
                f0, fs = starts[ci], SCHED[ci]
                t1 = p1.tile([K1, F_MAX], bf16, tag="xt1")
                t2 = p2.tile([K2, F_MAX], bf16, tag="xt2")
                nc.sync.dma_start(t1[:, :fs], xt[0:K1, f0 : f0 + fs])
                nc.scalar.dma_start(t2[:, :fs], xt[K1:IN_DIM, f0 : f0 + fs])
                return t1, t2

            nflush = 0

            def compute(ci, t1, t2):
                nonlocal ncopy, nflush
                f0, fs = starts[ci], SCHED[ci]
                otf = po.tile([128, F_MAX], bf16, tag="out")
                last = ci == len(SCHED) - 1
                # Output port mix: out tiles on partitions 64:128 hit only
                # odd SDMA ports; with t1 flat and t2 even-heavy the odd
                # ports 1,3 become the hottest (20 16KB-lines/chunk).  On
                # alternate chunks the two 4096-col pair-halves land on
                # partitions 0:64 / 64:128 (matmul col-group h0/h1) and
                # flush separately, spreading output bytes over BOTH port
                # parities; the 50/50 mix minimizes the max port load (18).
                flat = (not last) and ci % 2 == 0
                for g0 in range(0, fs, PGROUP):
                    gs = min(PGROUP, fs - g0)
                    g = g0 // PGROUP
                    par = (g // 2) % 2 if flat else 1
                    base = 64 * par
                    psf = pp.tile([128, PGROUP], f32, tag="ps")
                    ps = psf[base : base + 64, :]
                    # all K1 matmuls first, then all K2: fewer stationary
                    # switches; PE reorder hides background LDWEIGHTS
                    for b0 in range(0, gs, INNER):
                        bs = min(INNER, gs - b0)
                        nc.tensor.matmul(
                            ps[:, b0 : b0 + bs],
                            w1[:],
                            t1[:, g0 + b0 : g0 + b0 + bs],
                            start=True,
                            stop=False,
                        )
                    for b0 in range(0, gs, INNER):
                        bs = min(INNER, gs - b0)
                        nc.tensor.matmul(
                            ps[:, b0 : b0 + bs],
                            w2[:],
                            t2[:, g0 + b0 : g0 + b0 + bs],
                            start=False,
                            stop=True,
                        )
                    # PSUM drain + f32->bf16 downcast, alternating engines.
                    # Safe for the scalar ring only because the NEXT
                    # chunk's input DMAs were already emitted (see loop)
                    dst = otf[base : base + 64, g0 : g0 + gs]
                    if ncopy % 2 == 0:
                        nc.vector.tensor_copy(dst, ps[:, :gs])
                    else:
                        nc.scalar.copy(dst, ps[:, :gs])
                    ncopy += 1
                    if last:
                        # final chunk: flush per group on alternating rings
                        # so the drain tail overlaps the last copies
                        eng = nc.sync if g % 2 == 0 else nc.scalar
                        eng.dma_start(
                            out[:, f0 + g0 : f0 + g0 + gs],
                            otf[base : base + 64, g0 : g0 + gs],
                        )
                    elif flat and (g % 2 == 1 or g0 + gs >= fs):
                        # flush the completed pair from its partition half
                        c0 = (g // 2) * 2 * PGROUP
                        w = g0 + gs - c0
                        eng = nc.sync if nflush % 2 == 0 else nc.scalar
                        nflush += 1
                        eng.dma_start(
                            out[:, f0 + c0 : f0 + c0 + w],
                            otf[base : base + 64, c0 : c0 + w],
                        )
                if not last and not flat:
                    # classic whole-chunk flush from partitions 64:128;
                    # rings alternate to balance ring-serial transfer time
                    eng = nc.sync if nflush % 2 == 0 else nc.scalar
                    nflush += 1
                    eng.dma_start(out[:, f0 : f0 + fs], otf[64:128, :fs])

            # software-pipelined emission: loads run two chunks ahead of
            # compute so ring issues are never queued behind copy waits
            LOOKAHEAD = 2
            pend = [issue_loads(ci) for ci in range(LOOKAHEAD)]
            for ci in range(len(SCHED)):
                if ci + LOOKAHEAD < len(SCHED):
                    pend.append(issue_loads(ci + LOOKAHEAD))
                compute(ci, *pend.pop(0))
    nc.compile()
    return nc


def _run(in_maps, trace=False, **kw):
    from concourse.bass_utils import run_bass_kernel_spmd

    nc = _build()
    return run_bass_kernel_spmd(nc, in_maps, list(range(N_CORES)), trace=trace, **kw)


def _make_in_maps(x: np.ndarray, weight_: np.ndarray):
    import ml_dtypes

    bf16 = ml_dtypes.bfloat16
    wn = _w_norm(weight_).astype(bf16)
    xb = np.asarray(x, dtype=np.float32).astype(bf16)
    in_maps = []
    for i in range(N_CORES):
        xti = np.ascontiguousarray(xb[i * RPC : (i + 1) * RPC, :].T)
        in_maps.append({"xt": xti, "wn": wn})
    return in_maps


def kernel(x: np.ndarray, weight_: np.ndarray) -> np.ndarray:
    x = np.asarray(x)
    weight_ = np.asarray(weight_)
    res = _run(_make_in_maps(x, weight_)).results
    out_t = np.concatenate([res[i]["out_t"] for i in range(N_CORES)], axis=1)
    return np.ascontiguousarray(out_t.T).astype(np.float32)


# revision 55
# speedup vs baseline: 1.0722x; 1.0722x over previous
"""Trainium2 Bass kernel for nn_LyotFilter: out = x @ w_norm(weight_).

Strategy (data-parallel over 8 NeuronCores, memory-bound):
  - Host: compute the tiny [200, 64] normalized filter matrix in float32
    (mimicking the reference's f32 arithmetic), cast to bf16, and reshape
    each core's row-shard of x into a transposed, contiguous bf16
    [200, 62500] layout so the contraction dim lands on SBUF partitions
    with fully contiguous per-partition DMA.  bf16 wire format halves HBM
    traffic vs f32 (33 MB/core vs 66 MB); measured end-to-end absmax-rel
    error ~3.9e-3 (gate 2e-2).
  - Device (per core): stream xT chunks HBM->SBUF on the two HWDGE rings
    (t1 on sync, t2 on scalar), TensorE matmul (K split 128+72, PSUM fp32
    accumulate, 512-col blocks, 4 banks per PSUM tile), drain PSUM->SBUF
    with bf16 downcast alternating Vector/Scalar, and stream out.T back
    on the two rings, alternating, so ring-serial transfer time stays
    balanced.  Output placement alternates per chunk between partition
    halves 64:128 (odd SDMA ports) and a split pair layout on 0:64 +
    64:128 (both parities) -- the 50/50 mix minimizes the max per-port
    DMA byte load given that t1 covers all 16 ports while t2 (72
    partitions) is even-port-heavy.  Input DMAs are emitted two chunks
    ahead of compute so ring issues are never queued behind copy waits
    in an engine's FIFO.
  - Host: concatenate the 8 [64, 62500] results, transpose, cast f32.
"""

import functools

import numpy as np

N_CORES = 8
ROWS = 500000
RPC = ROWS // N_CORES  # 62500 rows per core
IN_DIM = 200
OUT_DIM = 64
K1 = 128               # first contraction chunk (partition limit)
K2 = IN_DIM - K1       # 72
PGROUP = 2048          # columns per PSUM tile (4 banks)
INNER = 512            # matmul moving free dim (one fp32 PSUM bank)
# chunk schedule: small leading chunks cut the pipeline-fill latency;
# the natural remainder tail shortens the drain (sums to RPC)
SCHED = [4096, 4096] + [8192] * 6 + [5156]
F_MAX = max(SCHED)
assert sum(SCHED) == RPC


def _w_norm(weight_: np.ndarray) -> np.ndarray:
    """[200, 64] filter matrix, float32 arithmetic mimicking the reference."""
    n = np.arange(220)
    skip = ((n >= 103) & (n <= 107)) | ((n >= 149) & (n <= 162)) | (n == 219)
    kept = n[~skip]
    bands = (400.0 + (2500.0 - 400.0) * kept / 220.0).astype(np.float32)
    num = np.float32(2.0 * np.pi * (-0.01))
    denom = weight_.astype(np.float32)[:, None] * (bands[None, :] * np.float32(1e-6))
    phase = (num / denom).astype(np.float32)
    w = np.float32(0.5) - np.float32(0.5) * np.cos(phase)
    wt = w.T  # [200, 64]
    wn = np.maximum(wt, np.float32(0.0)) / wt.sum(axis=0, dtype=np.float32)
    return np.ascontiguousarray(wn.astype(np.float32))


@functools.cache
def _build():
    import concourse.bass as bass
    import concourse.tile as tile
    from concourse import bacc, mybir

    f32 = mybir.dt.float32
    bf16 = mybir.dt.bfloat16
    nc = bacc.Bacc(
        "TRN2", target_bir_lowering=False, debug=False, num_devices=N_CORES
    )
    xt = nc.declare_dram_parameter("xt", [IN_DIM, RPC], bf16, isOutput=False)
    wn = nc.declare_dram_parameter("wn", [IN_DIM, OUT_DIM], bf16, isOutput=False)
    out = nc.declare_dram_parameter("out_t", [OUT_DIM, RPC], bf16, isOutput=True)

    starts = np.cumsum([0] + SCHED[:-1]).tolist()
    ncopy = 0

    with tile.TileContext(nc) as tc:
        with (
            tc.tile_pool(name="w", bufs=1) as wp,
            tc.tile_pool(name="xt1", bufs=4) as p1,
            tc.tile_pool(name="xt2", bufs=4) as p2,
            tc.tile_pool(name="outp", bufs=4) as po,
            tc.tile_pool(name="ps", bufs=2, space=bass.MemorySpace.PSUM) as pp,
        ):
            # weights ride SWDGE so the HWDGE rings start on x immediately
            w1 = wp.tile([K1, OUT_DIM], bf16, tag="w1")
            w2 = wp.tile([K2, OUT_DIM], bf16, tag="w2")
            nc.gpsimd.dma_start(w1[:], wn[0:K1, :])
            nc.gpsimd.dma_start(w2[:], wn[K1:IN_DIM, :])

            def issue_loads(ci):
                f0, fs = starts[ci], SCHED[ci]
                t1 = p1.tile([K1, F_MAX], bf16, tag="xt1")
                t2 = p2.tile([K2, F_MAX], bf16, tag="xt2")
                nc.sync.dma_start(t1[:, :fs], xt[0:K1, f0 : f0 + fs])
                nc.scalar.dma_start(t2[:, :fs], xt[K1:IN_DIM, f0 : f0 + fs])
                return t1, t2

            nflush = 0

            def compute(ci, t1, t2):
                nonlocal ncopy, nflush
                f0, fs = starts[ci], SCHED[ci]
                otf = po.tile([128, F_MAX], bf16, tag="out")
                last = ci == len(SCHED) - 1
                # Output port mix: out tiles on partitions 64:128 hit only
                # odd SDMA ports; with t1 flat and t2 even-heavy the odd
                # ports 1,3 become the hottest (20 16KB-lines/chunk).  On
                # alternate chunks the two 4096-col pair-halves land on
                # partitions 0:64 / 64:128 (matmul col-group h0/h1) and
                # flush separately, spreading output bytes over BOTH port
                # parities; the 50/50 mix minimizes the max port load (18).
                flat = (not last) and ci % 2 == 0
                for g0 in range(0, fs, PGROUP):
                    gs = min(PGROUP, fs - g0)
                    g = g0 // PGROUP
                    par = (g // 2) % 2 if flat else 1
                    base = 64 * par
                    psf = pp.tile([128, PGROUP], f32, tag="ps")
                    ps = psf[base : base + 64, :]
                    # all K1 matmuls first, then all K2: fewer stationary
                    # switches; PE reorder hides background LDWEIGHTS
                    for b0 in range(0, gs, INNER):
                        bs = min(INNER, gs - b0)
                        nc.tensor.matmul(
                            ps[:, b0 : b0 + bs],
                            w1[:],
                            t1[:, g0 + b0 : g0 + b0 + bs],
                            start=True,
                            stop=False,
                        )
                    for b0 in range(0, gs, INNER):
                        bs = min(INNER, gs - b0)
                        nc.tensor.matmul(
                            ps[:, b0 : b0 + bs],
                            w2[:],
                            t2[:, g0 + b0 : g0 + b0 + bs],
                            start=False,
                            stop=True,
                        )
                    # PSUM drain + f32->bf16 downcast, alternating engines.
                    # Safe for the scalar ring only because the NEXT
                    # chunk's input DMAs were already emitted (see loop)
                    dst = otf[base : base + 64, g0 : g0 + gs]
                    if ncopy % 2 == 0:
                        nc.vector.tensor_copy(dst, ps[:, :gs])
                    else:
                        nc.scalar.copy(dst, ps[:, :gs])
                    ncopy += 1
                    if last:
                        # final chunk: flush per group on alternating rings
                        # so the drain tail overlaps the last copies
                        eng = nc.sync if g % 2 == 0 else nc.scalar
                        eng.dma_start(
                            out[:, f0 + g0 : f0 + g0 + gs],
                            otf[base : base + 64, g0 : g0 + gs],
                        )
                    elif flat and (g % 2 == 1 or g0 + gs >= fs):
                        # flush the completed pair from its partition half
                        c0 = (g // 2) * 2 * PGROUP
                        w = g0 + gs - c0
                        eng = nc.sync if nflush % 2 == 0 else nc.scalar
                        nflush += 1
                        eng.dma_start(
                            out[:, f0 + c0 : f0 + c0 + w],
                            otf[base : base + 64, c0 : c0 + w],
                        )
                if not last and not flat:
                    # classic whole-chunk flush from partitions 64:128;
                    # rings alternate to balance ring-serial transfer time
                    eng = nc.sync if nflush % 2 == 0 else nc.scalar
                    nflush += 1
                    eng.dma_start(out[:, f0 : f0 + fs], otf[64:128, :fs])

            # software-pipelined emission: loads run two chunks ahead of
            # compute so ring issues are never queued behind copy waits
            LOOKAHEAD = 2
            pend = [issue_loads(ci) for ci in range(LOOKAHEAD)]
            for ci in range(len(SCHED)):
                if ci + LOOKAHEAD < len(SCHED):
                    pend.append(issue_loads(ci + LOOKAHEAD))
                compute(ci, *pend.pop(0))
    nc.compile()
    return nc


def _run(in_maps, trace=False, **kw):
    from concourse.bass_utils import run_bass_kernel_spmd

    nc = _build()
    return run_bass_kernel_spmd(nc, in_maps, list(range(N_CORES)), trace=trace, **kw)


def _make_in_maps(x: np.ndarray, weight_: np.ndarray):
    import ml_dtypes

    bf16 = ml_dtypes.bfloat16
    wn = _w_norm(weight_).astype(bf16)
    xb = np.asarray(x, dtype=np.float32).astype(bf16)
    in_maps = []
    for i in range(N_CORES):
        xti = np.ascontiguousarray(xb[i * RPC : (i + 1) * RPC, :].T)
        in_maps.append({"xt": xti, "wn": wn})
    return in_maps


def kernel(x: np.ndarray, weight_: np.ndarray) -> np.ndarray:
    x = np.asarray(x)
    weight_ = np.asarray(weight_)
    res = _run(_make_in_maps(x, weight_)).results
    out_t = np.concatenate([res[i]["out_t"] for i in range(N_CORES)], axis=1)
    return np.ascontiguousarray(out_t.T).astype(np.float32)
